# revision 12
# baseline (speedup 1.0000x reference)
"""Trainium2 Bass kernel for: out = x + softmax(x @ y^T) @ y  (per batch).

Full inputs: x, y: (8, 2048, 128) f32.  Data-parallel over batch: one batch
element per NeuronCore (8 cores).

Per-core algorithm (S=2048 seq, D=128 head dim, P=128 partitions):
  - Load x, y naturally; build xT, yT (d-major, float32r) via PE transposes.
  - For each i-chunk of 1024 (2 chunks), for each j-block of 128 (16 blocks):
      S^T[j, i] = yT_j.T @ xT_ic      (f32r matmuls, N=512, full PE rate)
      P^T = exp(S^T - 60)             (ACT, f32 PSUM -> bf16 SBUF)
      att_ext[i, 0:129] += P^T_ib.T @ [y_j | 1]   (bf16 matmuls, N=129)
    The appended ones column gives the softmax denominator for free.
  - out = x + att / l  (DVE reciprocal + fused scalar_tensor_tensor)

The constant -60 shift keeps exp in f32/bf16 range (scores reach ~±85 for
randn inputs at D=128) and is mathematically a no-op for softmax.

Emission order is software-pipelined: the PE runs S-matmuls of tile jb+1
before the att-matmuls of tile jb so it never stalls on the ACT exp; input
DMAs are chunked and the ic=1 transposes are spread through the ic=0 loop
to shorten the serial ramp-in.
"""

from contextlib import ExitStack

import numpy as np

import concourse.bass as bass
import concourse.mybir as mybir
import concourse.tile as tile
from concourse import bacc
from concourse.bass_utils import run_bass_kernel_spmd
from concourse.masks import make_identity

P = 128          # SBUF partitions
S = 2048         # sequence length (both x and y)
D = 128          # feature dim
NB = S // P      # 16 blocks of 128 rows
IC = 1024        # i-chunk width processed per inner pipeline
NIC = S // IC    # 2 chunks
IBC = IC // P    # 8 i-blocks per chunk
AW = D + 1       # att output width incl. ones column (129)
SHIFT = -60.0    # exp bias; no-op for softmax, keeps exp() in range
DMA_CH = 4       # input DMA chunks per tensor

F32 = mybir.dt.float32
F32R = mybir.dt.float32r
BF16 = mybir.dt.bfloat16

N_CORES = 8
B = 8


def build_module(reps: int = 1):
    """Build + compile the single-core Bass module (one batch element).

    reps>1 wraps the computation in a hardware loop — benchmarking only.
    """
    nc = bacc.Bacc("TRN2", target_bir_lowering=False, debug=False)
    x_d = nc.dram_tensor("x", [S, D], F32, kind="ExternalInput").ap()
    y_d = nc.dram_tensor("y", [S, D], F32, kind="ExternalInput").ap()
    o_d = nc.dram_tensor("out", [S, D], F32, kind="ExternalOutput").ap()

    # DRAM viewed as (block, partition, d) so each DMA line is contiguous.
    x_v = x_d.rearrange("(t p) d -> p t d", p=P)
    y_v = y_d.rearrange("(t p) d -> p t d", p=P)
    o_v = o_d.rearrange("(t p) d -> p t d", p=P)

    with tile.TileContext(nc) as tc, ExitStack() as ctx:
        sing = ctx.enter_context(tc.tile_pool(name="sing", bufs=1))
        ppool = ctx.enter_context(tc.tile_pool(name="pt", bufs=3))
        fpool = ctx.enter_context(tc.tile_pool(name="fin", bufs=2))
        rpool = ctx.enter_context(tc.tile_pool(name="rcp", bufs=4))
        # PSUM: "s" tiles (128,1024) f32 = 2 banks, bufs=2 -> 4 banks.
        # "att" tiles (128,512) f32 = 1 bank, bufs=3 -> 3 banks.  Total 7/8.
        ps_s = ctx.enter_context(tc.tile_pool(name="ps_s", bufs=2, space="PSUM"))
        ps_att = ctx.enter_context(tc.tile_pool(name="ps_att", bufs=3, space="PSUM"))

        ident = sing.tile([P, P], F32)
        make_identity(nc, ident)
        bias_t = sing.tile([P, 1], F32)
        nc.vector.memset(bias_t, SHIFT)

        def body():
            x_nat = sing.tile([P, NB, D], F32)
            y_nat = sing.tile([P, NB, D], F32)
            nchunk = NB // DMA_CH
            for c in range(DMA_CH):
                cs = slice(c * nchunk, (c + 1) * nchunk)
                nc.sync.dma_start(out=y_nat[:, cs, :], in_=y_v[:, cs, :])
            for c in range(DMA_CH):
                cs = slice(c * nchunk, (c + 1) * nchunk)
                nc.sync.dma_start(out=x_nat[:, cs, :], in_=x_v[:, cs, :])

            # d-major (f32r) copies via PE transpose; PSUM->SBUF evacuation
            # split across DVE (x) and ACT (y) so prep runs two copy streams.
            xT = sing.tile([P, NB, P], F32R)  # [d, t, i_local]
            yT = sing.tile([P, NB, P], F32R)  # [d, t, j_local]

            def transpose_block(dst, src_nat, t, who):
                pt_ = ps_s.tile([P, P], F32, tag="s", name=f"tp_{who}_{t}")
                nc.tensor.transpose(pt_, src_nat[:, t, :], ident)
                if who == "y":
                    nc.scalar.copy(dst[:, t, :], pt_)
                else:
                    nc.vector.tensor_copy(dst[:, t, :], pt_)

            for t in range(NB):
                transpose_block(yT, y_nat, t, "y")
                if t < IBC:
                    transpose_block(xT, x_nat, t, "x")

            # bf16 [y | 1] blocks for the att matmul rhs (GPSIMD: idle engine).
            y_ext = sing.tile([P, NB, AW], BF16)
            nc.gpsimd.tensor_copy(y_ext[:, :, 0:D], y_nat)
            nc.gpsimd.memset(y_ext[:, :, D:AW], 1.0)

            def emit_s_matmuls(ic, jb):
                s_t = ps_s.tile([P, IC], F32, tag="s", name=f"s_{ic}_{jb}")
                for h in range(IC // 512):
                    blk = ic * IBC + h * 4
                    nc.tensor.matmul(
                        s_t[:, h * 512:(h + 1) * 512],
                        lhsT=yT[:, jb, :],
                        rhs=xT[:, blk:blk + 4, :],
                        start=True,
                        stop=True,
                    )
                return s_t

            def emit_exp(s_t, ic, jb):
                p_t = ppool.tile([P, IC], BF16, tag="p", name=f"p_{ic}_{jb}")
                nc.scalar.activation(
                    p_t, s_t, mybir.ActivationFunctionType.Exp,
                    bias=bias_t[:, 0:1], scale=1.0,
                )
                return p_t

            def emit_att_matmuls(att_tiles, p_t, jb):
                for ib in range(IBC):
                    bank, slot = divmod(ib, 3)
                    off = slot * AW
                    nc.tensor.matmul(
                        att_tiles[bank][:, off:off + AW],
                        lhsT=p_t[:, ib * P:(ib + 1) * P],
                        rhs=y_ext[:, jb, :],
                        start=(jb == 0 and slot == 0),
                        stop=(jb == NB - 1 and (slot == 2 or ib == IBC - 1)),
                    )

            for ic in range(NIC):
                att_tiles = [
                    ps_att.tile([P, 512], F32, tag="att", name=f"att_{ic}_{k}")
                    for k in range(3)
                ]
                prev_p = None
                for jb in range(NB):
                    s_t = emit_s_matmuls(ic, jb)
                    # spread the ic=1 x-transposes through the ic=0 loop
                    if ic == 0 and jb < IBC:
                        transpose_block(xT, x_nat, IBC + jb, "x")
                    if prev_p is not None:
                        emit_att_matmuls(att_tiles, prev_p, jb - 1)
                    prev_p = emit_exp(s_t, ic, jb)
                emit_att_matmuls(att_tiles, prev_p, NB - 1)

                # Normalize + residual add + store.
                o_t = fpool.tile([P, IBC, D], F32, tag="o", name=f"o_{ic}")
                for ib in range(IBC):
                    bank, slot = divmod(ib, 3)
                    off = slot * AW
                    r_t = rpool.tile([P, 1], F32, tag="r", name=f"r_{ic}_{ib}")
                    nc.vector.reciprocal(
                        r_t, att_tiles[bank][:, off + D:off + AW])
                    nc.vector.scalar_tensor_tensor(
                        o_t[:, ib, :],
                        in0=att_tiles[bank][:, off:off + D],
                        scalar=r_t,
                        in1=x_nat[:, ic * IBC + ib, :],
                        op0=mybir.AluOpType.mult,
                        op1=mybir.AluOpType.add,
                    )
                nc.sync.dma_start(
                    out=o_v[:, ic * IBC:(ic + 1) * IBC, :], in_=o_t
                )

        if reps == 1:
            body()
        else:
            with tc.For_i(0, reps, 1, hint_engines=(
                    mybir.EngineType.PE, mybir.EngineType.Activation,
                    mybir.EngineType.DVE)):
                body()

    nc.compile()
    return nc


_NC = None


def _get_module():
    global _NC
    if _NC is None:
        _NC = build_module()
    return _NC


def kernel(x: np.ndarray, y: np.ndarray, trace: bool = False):
    """Full-input entry point: x, y (8, 2048, 128) f32 -> (8, 2048, 128) f32."""
    x = np.asarray(x)
    y = np.asarray(y)
    assert x.shape == (B, S, D) and y.shape == (B, S, D)
    nc = _get_module()
    in_maps = [
        {"x": np.ascontiguousarray(x[b], dtype=np.float32),
         "y": np.ascontiguousarray(y[b], dtype=np.float32)}
        for b in range(B)
    ]
    res = run_bass_kernel_spmd(nc, in_maps, core_ids=list(range(N_CORES)),
                               trace=trace)
    out = np.stack([res.results[b]["out"] for b in range(B)], axis=0)
    if trace:
        return out, res
    return out
